# revision 1
# baseline (speedup 1.0000x reference)
"""DeepSeek-V3-style MoE layer on 8 Trainium2 NeuronCores.

Sharding: expert-parallel — core c owns routed experts {2c, 2c+1} and a
128-wide slice of the shared expert's intermediate dim. Every core sees all
2048 tokens, computes the router (fp32, on device), evaluates its two
experts densely over all tokens (bf16 matmuls, fp32 accumulate), scales by
the combine weights, adds its shared-expert partial, and writes a partial
[T, H] output. The host sums the 8 partials.

Device data layouts (per core):
  xT    [H, T]      fp32  hidden states, transposed (feature-on-partition)
  wrT   [H, E]      fp32  router weight, transposed
  ebias [128, E]    fp32  e_score_correction_bias broadcast across partitions
  selv  [128, 2, E] fp32  one-hot selectors for this core's two experts
  gwT/uwT [2, H, I] bf16  gate/up weights, transposed, this core's experts
  dwT   [2, I, H]   bf16  down weights, transposed
  shgT/shuT [H, 128] bf16 shared gate/up rows for this core's I_sh slice
  shdT  [128, H]    bf16  shared down cols for this core's I_sh slice
  y     [T, H]      fp32  partial output (token-on-partition)
"""

import sys

sys.path.insert(0, "/opt/trn_rl_repo")

import numpy as np
import ml_dtypes

import concourse.bacc as bacc
import concourse.mybir as mybir
import concourse.tile as tile
from concourse.bass import ts
from concourse.bass_utils import run_bass_kernel_spmd
from concourse.masks import make_identity

B, S, H = 1, 2048, 1024
T = B * S
E, K = 16, 4
G = 4
I_MOE = 512
I_SH = 2 * I_MOE
SCALE = 2.5
NCORES = 8
E_LOC = E // NCORES      # 2 experts per core
ISH_LOC = I_SH // NCORES  # 128 shared-intermediate rows per core

KT = H // 128    # 8 contraction tiles over H
IT = I_MOE // 128  # 4 tiles over I
TT = T // 128    # 16 token tiles of 128
T4 = T // 512    # 4 token tiles of 512
HH = H // 512    # 2 output halves

f32 = mybir.dt.float32
bf16 = mybir.dt.bfloat16
AF = mybir.ActivationFunctionType
ALU = mybir.AluOpType
AX = mybir.AxisListType

NEG = -1.0e30


def build_kernel(loop_iters=1, loop_scope="all", do_route=True, do_gu=True, do_down=True):
    nc = bacc.Bacc(None, target_bir_lowering=False)
    xT = nc.dram_tensor("xT", [H, T], f32, kind="ExternalInput")
    xTb = nc.dram_tensor("xTb", [H, T], bf16, kind="ExternalInput")
    wrT = nc.dram_tensor("wrT", [H, E], f32, kind="ExternalInput")
    ebias = nc.dram_tensor("ebias", [128, E], f32, kind="ExternalInput")
    selv = nc.dram_tensor("selv", [128, E_LOC, E], f32, kind="ExternalInput")
    gwT = nc.dram_tensor("gwT", [E_LOC, H, I_MOE], bf16, kind="ExternalInput")
    uwT = nc.dram_tensor("uwT", [E_LOC, H, I_MOE], bf16, kind="ExternalInput")
    dwT = nc.dram_tensor("dwT", [E_LOC, I_MOE, H], bf16, kind="ExternalInput")
    shgT = nc.dram_tensor("shgT", [H, ISH_LOC], bf16, kind="ExternalInput")
    shuT = nc.dram_tensor("shuT", [H, ISH_LOC], bf16, kind="ExternalInput")
    shdT = nc.dram_tensor("shdT", [ISH_LOC, H], bf16, kind="ExternalInput")
    y = nc.dram_tensor("y", [T, H], f32, kind="ExternalOutput")

    xT_r = xT.ap().rearrange("(ko p) t -> p ko t", p=128)
    xTb_r = xTb.ap().rearrange("(ko p) t -> p ko t", p=128)
    y_r = y.ap().rearrange("(tt p) h -> p tt h", p=128)

    with tile.TileContext(nc) as tc:
        with (
            tc.tile_pool(name="consts", bufs=1) as consts,
            tc.tile_pool(name="wpool", bufs=1) as wpool,
            tc.tile_pool(name="xbpool", bufs=1) as xbpool,
            tc.tile_pool(name="route", bufs=1) as route,
            tc.tile_pool(name="xfpool", bufs=4) as xfpool,
            tc.tile_pool(name="hpool", bufs=4) as hpool,
            tc.tile_pool(name="opool", bufs=4) as opool,
            tc.tile_pool(name="pp", bufs=8, space="PSUM") as pp,
        ):
            loop_cm = None
            if loop_iters > 1 and loop_scope == "all":
                loop_cm = tc.For_i(0, loop_iters, 1)
                loop_cm.__enter__()
            # ---- constants ----
            ident = consts.tile([128, 128], f32)
            make_identity(nc, ident[:])
            wr_sb = consts.tile([128, KT, E], f32)
            nc.sync.dma_start(wr_sb[:], wrT.ap().rearrange("(ko p) e -> p ko e", p=128))
            bias_sb = consts.tile([128, E], f32)
            nc.sync.dma_start(bias_sb[:], ebias.ap())
            sel_sb = consts.tile([128, E_LOC, E], f32)
            nc.sync.dma_start(sel_sb[:], selv.ap())
            ones_sb = consts.tile([1, 128], f32)
            nc.any.memset(ones_sb[:], 1.0)

            # ---- load x (bf16 direct for experts, fp32 for the router) ----
            x_b = xbpool.tile([128, KT, T], bf16)
            scT = route.tile([16, T4, 512], f32)  # sigmoid scores, expert-on-partition
            for t in range(T4):
                nc.sync.dma_start(x_b[:, :, ts(t, 512)], xTb_r[:, :, ts(t, 512)])
                if not do_route:
                    continue
                ps_sc = pp.tile([128, 512], f32, tag="bank", name=f"ps_sc{t}")[:16, :]
                for k in range(KT):
                    x_f = xfpool.tile([128, 512], f32, tag="xf", name=f"xf{t}_{k}")
                    nc.sync.dma_start(x_f[:], xT_r[:, k, ts(t, 512)])
                    nc.tensor.matmul(
                        ps_sc[:], wr_sb[:, k, :], x_f[:],
                        start=(k == 0), stop=(k == KT - 1),
                    )
                nc.scalar.activation(scT[:, t, :], ps_sc[:], AF.Sigmoid)

            # ---- weights ----
            gw_sb = wpool.tile([128, E_LOC, KT, I_MOE], bf16)
            uw_sb = wpool.tile([128, E_LOC, KT, I_MOE], bf16)
            dw_sb = wpool.tile([128, E_LOC, IT, H], bf16)
            for e in range(E_LOC):
                nc.sync.dma_start(
                    gw_sb[:, e], gwT.ap()[e].rearrange("(ko p) i -> p ko i", p=128)
                )
                nc.sync.dma_start(
                    uw_sb[:, e], uwT.ap()[e].rearrange("(ko p) i -> p ko i", p=128)
                )
                nc.sync.dma_start(
                    dw_sb[:, e], dwT.ap()[e].rearrange("(ko p) h -> p ko h", p=128)
                )
            shg_sb = wpool.tile([128, KT, ISH_LOC], bf16)
            shu_sb = wpool.tile([128, KT, ISH_LOC], bf16)
            shd_sb = wpool.tile([128, H], bf16)
            nc.sync.dma_start(shg_sb[:], shgT.ap().rearrange("(ko p) i -> p ko i", p=128))
            nc.sync.dma_start(shu_sb[:], shuT.ap().rearrange("(ko p) i -> p ko i", p=128))
            nc.sync.dma_start(shd_sb[:], shdT.ap())

            cb = route.tile([128, E_LOC, T], f32)
            Cloc = route.tile([128, E_LOC, TT], f32)
            if not do_route:
                nc.gpsimd.memset(cb[:], 0.5)
            if do_route:
                # ---- transpose scores to token-on-partition: sc [128, TT, E] ----
                sc = route.tile([128, TT, E], f32)
                for tt in range(TT):
                    ps_tr = pp.tile([128, 512], f32, tag="bank", name=f"ps_tr{tt}")[:, :16]
                    nc.tensor.transpose(
                        ps_tr[:], scT[:, tt // 4, ts(tt % 4, 128)], ident[:16, :16]
                    )
                    nc.vector.tensor_copy(sc[:, tt, :], ps_tr[:])

                # ---- routing: group-limited top-4 combine weights, all in fp32 ----
                scb = route.tile([128, TT, E], f32)
                nc.vector.tensor_tensor(
                    scb[:], sc[:], bias_sb[:, None, :].to_broadcast([128, TT, E]), ALU.add
                )
                scb4 = scb[:].rearrange("p t (g e) -> p t g e", g=G)
                # per-group top-2 sum
                m1 = route.tile([128, TT, G], f32)
                nc.vector.tensor_reduce(m1[:], scb4, axis=AX.X, op=ALU.max)
                eq4 = route.tile([128, TT, G, G], f32)
                nc.vector.tensor_tensor(
                    eq4[:], scb4, m1[:, :, :, None].to_broadcast([128, TT, G, G]), ALU.is_ge
                )
                tmp4 = route.tile([128, TT, G, G], f32)
                nc.vector.scalar_tensor_tensor(tmp4[:], eq4[:], NEG, scb4, ALU.mult, ALU.add)
                gs = m1
                m2 = route.tile([128, TT, G], f32)
                nc.vector.tensor_reduce(m2[:], tmp4[:], axis=AX.X, op=ALU.max)
                nc.vector.tensor_tensor(gs[:], m1[:], m2[:], ALU.add)
                # top-2 groups -> threshold tg (2nd-largest group score)
                gm1 = route.tile([128, TT], f32)
                nc.vector.tensor_reduce(gm1[:], gs[:], axis=AX.X, op=ALU.max)
                eqg = route.tile([128, TT, G], f32)
                nc.vector.tensor_tensor(
                    eqg[:], gs[:], gm1[:, :, None].to_broadcast([128, TT, G]), ALU.is_ge
                )
                tmpg = route.tile([128, TT, G], f32)
                nc.vector.scalar_tensor_tensor(tmpg[:], eqg[:], NEG, gs[:], ALU.mult, ALU.add)
                tg = route.tile([128, TT], f32)
                nc.vector.tensor_reduce(tg[:], tmpg[:], axis=AX.X, op=ALU.max)
                gmask = eqg
                nc.vector.tensor_tensor(
                    gmask[:], gs[:], tg[:, :, None].to_broadcast([128, TT, G]), ALU.is_ge
                )
                # mask scores outside the chosen groups (masked value = 0, as reference)
                sm = route.tile([128, TT, E], f32)
                sm4 = sm[:].rearrange("p t (g e) -> p t g e", g=G)
                nc.vector.tensor_tensor(
                    sm4, scb4, gmask[:, :, :, None].to_broadcast([128, TT, G, G]), ALU.mult
                )
                # top-4 threshold over the masked scores
                cur = sm
                for r in range(K - 1):
                    rmax = route.tile([128, TT], f32, tag="rmax", name=f"rmax{r}")
                    nc.vector.tensor_reduce(rmax[:], cur[:], axis=AX.X, op=ALU.max)
                    eqt = route.tile([128, TT, E], f32, tag="eqt", name=f"eqt{r}")
                    nc.vector.tensor_tensor(
                        eqt[:], cur[:], rmax[:, :, None].to_broadcast([128, TT, E]), ALU.is_ge
                    )
                    nxt = route.tile([128, TT, E], f32, tag=f"nxt{r % 2}", name=f"nxt{r}")
                    nc.vector.scalar_tensor_tensor(
                        nxt[:], eqt[:], NEG, cur[:], ALU.mult, ALU.add
                    )
                    cur = nxt
                t4 = route.tile([128, TT], f32)
                nc.vector.tensor_reduce(t4[:], cur[:], axis=AX.X, op=ALU.max)
                selm = route.tile([128, TT, E], f32)
                nc.vector.tensor_tensor(
                    selm[:], sm[:], t4[:, :, None].to_broadcast([128, TT, E]), ALU.is_ge
                )
                # weights from unbiased scores; normalize; scale
                w = selm
                nc.vector.tensor_tensor(w[:], sc[:], selm[:], ALU.mult)
                den = route.tile([128, TT], f32)
                nc.vector.tensor_reduce(den[:], w[:], axis=AX.X, op=ALU.add)
                nc.vector.tensor_scalar_add(den[:], den[:], 1e-20)
                rec = route.tile([128, TT], f32)
                nc.vector.reciprocal(rec[:], den[:])
                C = route.tile([128, TT, E], f32)
                nc.vector.scalar_tensor_tensor(
                    C[:], w[:], SCALE, rec[:, :, None].to_broadcast([128, TT, E]),
                    ALU.mult, ALU.mult,
                )
                # per-local-expert combine columns: Cloc[:, e, tt]
                for e in range(E_LOC):
                    tmpsel = route.tile([128, TT, E], f32, tag="tmpsel")
                    nc.vector.tensor_tensor(
                        tmpsel[:], C[:],
                        sel_sb[:, e, None, :].to_broadcast([128, TT, E]), ALU.mult,
                    )
                    nc.vector.tensor_reduce(
                        Cloc[:, e, :], tmpsel[:], axis=AX.X, op=ALU.add
                    )
                # broadcast combine weights to feature-partition layout:
                # transpose [128,TT] -> [TT,128] on PE, flatten to one row by
                # DMA, then replicate across partitions on GPSIMD.
                for e in range(E_LOC):
                    ps_ct = pp.tile([128, 512], f32, tag="bank", name=f"ps_ct{e}")
                    nc.tensor.transpose(
                        ps_ct[:TT, :128], Cloc[:, e, :], ident[:]
                    )
                    ct_sb = route.tile([TT, 128], f32, tag="ct", name=f"ct{e}")
                    nc.vector.tensor_copy(ct_sb[:], ps_ct[:TT, :128])
                    # flatten [16 partitions x 128] to one [1, 2048] row (DMA
                    # reads partition-major), then replicate across partitions
                    # with K=1 ones-matmuls.
                    row = route.tile([1, T], f32, tag="row", name=f"row{e}")
                    nc.sync.dma_start(row[0:1, :], ct_sb[:, :])
                    for t in range(T4):
                        cb_ps = pp.tile([128, 512], f32, tag="bank",
                                        name=f"cb_ps{e}_{t}")
                        nc.tensor.matmul(
                            cb_ps[:], ones_sb[:], row[:, ts(t, 512)],
                            start=True, stop=True,
                        )
                        nc.vector.tensor_copy(cb[:, e, ts(t, 512)], cb_ps[:])


            # ---- main dense expert compute ----
            if loop_iters > 1 and loop_scope == "main":
                loop_cm = tc.For_i(0, loop_iters, 1)
                loop_cm.__enter__()
            for t in range(T4):
                tsl = ts(t, 512)
                h_e = []
                for e in range(E_LOC):
                    h = hpool.tile([128, IT, 512], bf16, tag=f"h{e}", name=f"h{e}_{t}")
                    if not do_gu:
                        nc.any.memset(h[:], 0.0)
                        h_e.append(h)
                        continue
                    for i in range(IT):
                        gp = pp.tile([128, 512], f32, tag="bank", name=f"gp{t}_{e}_{i}")
                        for k in range(KT):
                            nc.tensor.matmul(
                                gp[:], gw_sb[:, e, k, ts(i, 128)], x_b[:, k, tsl],
                                start=(k == 0), stop=(k == KT - 1),
                            )
                        up = pp.tile([128, 512], f32, tag="bank", name=f"up{t}_{e}_{i}")
                        for k in range(KT):
                            nc.tensor.matmul(
                                up[:], uw_sb[:, e, k, ts(i, 128)], x_b[:, k, tsl],
                                start=(k == 0), stop=(k == KT - 1),
                            )
                        s_sb = opool.tile([128, 512], bf16, tag="s", name=f"s{t}_{e}_{i}")
                        nc.scalar.activation(s_sb[:], gp[:], AF.Silu)
                        nc.vector.tensor_tensor(h[:, i, :], s_sb[:], up[:], ALU.mult)
                        nc.vector.tensor_tensor(
                            h[:, i, :], h[:, i, :], cb[:, e, tsl], ALU.mult
                        )
                    h_e.append(h)
                hs = hpool.tile([128, 512], bf16, tag="hsh", name=f"hs_{t}")
                if not do_gu:
                    nc.any.memset(hs[:], 0.0)
                gp = None
                if do_gu:
                    gp = pp.tile([128, 512], f32, tag="bank", name=f"gps_{t}")
                if do_gu:
                    for k in range(KT):
                        nc.tensor.matmul(
                            gp[:], shg_sb[:, k, :], x_b[:, k, tsl],
                            start=(k == 0), stop=(k == KT - 1),
                        )
                    up = pp.tile([128, 512], f32, tag="bank", name=f"ups_{t}")
                    for k in range(KT):
                        nc.tensor.matmul(
                            up[:], shu_sb[:, k, :], x_b[:, k, tsl],
                            start=(k == 0), stop=(k == KT - 1),
                        )
                    s_sb = opool.tile([128, 512], bf16, tag="s", name=f"ss_{t}")
                    nc.scalar.activation(s_sb[:], gp[:], AF.Silu)
                    nc.vector.tensor_tensor(hs[:], s_sb[:], up[:], ALU.mult)

                # down-projection in token-on-partition layout + combine
                if not do_down:
                    continue
                for c in range(4):
                    tt = t * 4 + c
                    csl = ts(c, 128)
                    for half in range(HH):
                        hsl = ts(half, 512)
                        yp = pp.tile([128, 512], f32, tag="bank", name=f"yp_{tt}_{half}")
                        for e in range(E_LOC):
                            for k in range(IT):
                                nc.tensor.matmul(
                                    yp[:], h_e[e][:, k, csl], dw_sb[:, e, k, hsl],
                                    start=(e == 0 and k == 0), stop=False,
                                )
                        nc.tensor.matmul(
                            yp[:], hs[:, csl], shd_sb[:, hsl], start=False, stop=True
                        )
                        out0 = opool.tile([128, 512], f32, tag="out", name=f"o_{tt}_{half}")
                        nc.vector.tensor_copy(out0[:], yp[:])
                        nc.sync.dma_start(y_r[:, tt, hsl], out0[:])

            if loop_iters > 1:
                loop_cm.__exit__(None, None, None)

    nc.compile()
    return nc


_NC_CACHE = {}


def _get_nc():
    if "nc" not in _NC_CACHE:
        _NC_CACHE["nc"] = build_kernel()
    return _NC_CACHE["nc"]


def make_in_maps(hidden_states, router_weight, e_bias, gate_w, up_w, down_w,
                 sh_gate_w, sh_up_w, sh_down_w):
    bf = ml_dtypes.bfloat16
    x = np.asarray(hidden_states, np.float32).reshape(T, H)
    xT_np = np.ascontiguousarray(x.T)
    xTb_np = xT_np.astype(bf)
    wrT_np = np.ascontiguousarray(np.asarray(router_weight, np.float32).T)
    eb = np.asarray(e_bias, np.float32)
    ebias_np = np.ascontiguousarray(np.broadcast_to(eb[None, :], (128, E)))
    gate_w = np.asarray(gate_w, np.float32)
    up_w = np.asarray(up_w, np.float32)
    down_w = np.asarray(down_w, np.float32)
    sh_gate_w = np.asarray(sh_gate_w, np.float32)
    sh_up_w = np.asarray(sh_up_w, np.float32)
    sh_down_w = np.asarray(sh_down_w, np.float32)

    in_maps = []
    for c in range(NCORES):
        es = [E_LOC * c + j for j in range(E_LOC)]
        sel = np.zeros((E_LOC, E), np.float32)
        for j, e in enumerate(es):
            sel[j, e] = 1.0
        selv_np = np.ascontiguousarray(
            np.broadcast_to(sel[None], (128, E_LOC, E))
        )
        gwT_np = np.ascontiguousarray(
            np.transpose(gate_w[es], (0, 2, 1))
        ).astype(bf)
        uwT_np = np.ascontiguousarray(np.transpose(up_w[es], (0, 2, 1))).astype(bf)
        dwT_np = np.ascontiguousarray(np.transpose(down_w[es], (0, 2, 1))).astype(bf)
        rsl = slice(ISH_LOC * c, ISH_LOC * (c + 1))
        shgT_np = np.ascontiguousarray(sh_gate_w[rsl, :].T).astype(bf)
        shuT_np = np.ascontiguousarray(sh_up_w[rsl, :].T).astype(bf)
        shdT_np = np.ascontiguousarray(sh_down_w[:, rsl].T).astype(bf)
        in_maps.append({
            "xT": xT_np,
            "xTb": xTb_np,
            "wrT": wrT_np,
            "ebias": ebias_np,
            "selv": selv_np,
            "gwT": gwT_np,
            "uwT": uwT_np,
            "dwT": dwT_np,
            "shgT": shgT_np,
            "shuT": shuT_np,
            "shdT": shdT_np,
        })
    return in_maps


def run(in_maps, **kwargs):
    nc = _get_nc()
    return run_bass_kernel_spmd(nc, in_maps, core_ids=list(range(NCORES)), **kwargs)


def kernel(hidden_states, router_weight, e_bias, gate_w, up_w, down_w,
           sh_gate_w, sh_up_w, sh_down_w):
    in_maps = make_in_maps(hidden_states, router_weight, e_bias, gate_w, up_w,
                           down_w, sh_gate_w, sh_up_w, sh_down_w)
    res = run(in_maps)
    out = np.zeros((T, H), np.float32)
    for c in range(NCORES):
        out += res.results[c]["y"]
    return out.reshape(B, S, H).astype(np.float32)

